# revision 1
# baseline (speedup 1.0000x reference)
"""Trainium2 Bass kernel for nn_ContextEncoder (GRU feature encoder + DenseGAT readout).

Contract: kernel(**inputs) takes the FULL unsharded inputs (numpy, as produced
by setup_inputs) and returns the FULL output [B, CD] float32.

Strategy: data-parallel over the batch axis B across 8 NeuronCores. Each core
processes 16 batches = 2048 (batch, node) rows:
  - feature pipeline (speed + turn-angle) on device
  - 127-step GRU (hidden 128) with bf16 matmuls and fp32 PSUM accumulation
  - dense-GAT readout reduced analytically to small matmuls (only node 0 of
    the attention output is needed, and the per-head linear map commutes with
    the attention-weighted sum).
"""

import sys

sys.path.insert(0, "/opt/trn_rl_repo")

import numpy as np
import ml_dtypes

import concourse.bass as bass
import concourse.bacc as bacc
import concourse.mybir as mybir
import concourse.tile as tile
from concourse.bass_utils import run_bass_kernel_spmd

F32 = mybir.dt.float32
BF16 = mybir.dt.bfloat16
AF = mybir.ActivationFunctionType
ALU = mybir.AluOpType
AX = mybir.AxisListType

N_CORES = 8
B, N, L, HID, CD, HEADS = 128, 128, 128, 128, 128, 4
T = L - 1  # 127 GRU steps
BC = B // N_CORES  # batches per core = 16
R = BC * N  # rows per core = 2048
EPS = 1e-6
NEG_SLOPE = 0.2

# Abramowitz & Stegun 4.4.45: arccos(x) ~= sqrt(1-x) * poly(x), 0<=x<=1,
# |err| <= 6.7e-5 rad.
AC0, AC1, AC2, AC3 = 1.5707288, -0.2121144, 0.0742610, -0.0187293

NSTREAM = 4
SC = R // NSTREAM  # 512 rows per stream chunk
PACK_PAIRS = True  # row-group pack ih/bias matmuls across stream pairs


def _build_program(repeats=1, t_steps=T, skip_gru=False, skip_gat=False):
    nc = bacc.Bacc("TRN2", target_bir_lowering=False, debug=False,
                   num_devices=N_CORES)

    # Per-core inputs (already sharded/laid out by the host wrapper).
    xr_d = nc.dram_tensor("xr", [R, 2 * L], F32, kind="ExternalInput")
    whhT_d = nc.dram_tensor("whhT", [HID, 3 * HID], BF16, kind="ExternalInput")
    # ih lhsT (rows bias/wv/wa) replicated at partition bases {0, 32} for
    # pair-wise row-group packing; bhh_n likewise at {0, 32}.
    wih_d = nc.dram_tensor("wih_aug", [35, 3 * HID], BF16, kind="ExternalInput")
    bhhn_d = nc.dram_tensor("bhh_n", [33, HID], BF16, kind="ExternalInput")
    ident_d = nc.dram_tensor("ident", [128, 128], BF16, kind="ExternalInput")
    uwd_d = nc.dram_tensor("uwd", [HID, 2 * HEADS], BF16, kind="ExternalInput")
    wgT_d = nc.dram_tensor("wgT", [HID, HEADS * CD], BF16, kind="ExternalInput")
    gbias_d = nc.dram_tensor("gbias", [1, CD], BF16, kind="ExternalInput")
    out_d = nc.dram_tensor("out", [BC, CD], F32, kind="ExternalOutput")

    NT = R // 128  # 16 row tiles
    with tile.TileContext(nc) as tc:
        with (
            tc.tile_pool(name="dram", bufs=1, space="DRAM") as dpool,
            tc.tile_pool(name="const", bufs=1) as cpool,
        ):
            f3 = dpool.tile([T, 3, R], BF16)  # per-step rhs rows (v, ang, 1)
            ident = cpool.tile([128, 128], BF16, tag="ident")
            nc.sync.dma_start(ident[:], ident_d.ap())
            ones = cpool.tile([1, R], BF16, tag="ones")
            nc.vector.memset(ones[:], 1.0)
            for _ in range(repeats):
                _build_features(nc, tc, xr_d, f3, NT, ident)
                if not skip_gru:
                    _build_gru_gat(nc, tc, f3, whhT_d, wih_d, bhhn_d, ident,
                                   ones, uwd_d, wgT_d, gbias_d, out_d,
                                   t_steps, skip_gat)

    nc.compile()
    return nc


def _build_features(nc, tc, xr_d, f3, NT, ident):
    """v[t] = |x[t+1]-x[t]|, ang[t] = arccos(clip(pv*v/((pv+eps)(v+eps)))).

    The speeds are nonnegative scalars, so cos is ~1-eps*(pv+v)/(pv*v) and
    the angle is tiny; arccos(c) = sqrt(2(1-c))*(1+O(1-c)) is exact to
    ~5e-6 rad here, with 1-c = eps*(pv+v+eps)/((pv+eps)(v+eps)) computed
    directly (always >= 0, no clipping needed). pv is v shifted one step,
    realized with strided views instead of copies.

    Layout: rows on partitions (16 tiles of 128), t on free (127).
    Ends by transposing to [t, row] and DMAing into f3 DRAM [T, 3, R].
    """
    xr = xr_d.ap()  # [R, 2L] flat, contiguous per row

    with (
        tc.tile_pool(name="feat_in", bufs=1) as fin,
        tc.tile_pool(name="feat_keep", bufs=1) as fkeep,
        tc.tile_pool(name="feat_ps", bufs=3, space="PSUM") as fps,
    ):
        xall = fin.tile([128, NT * 2 * L], F32, tag="xall")
        # one strided DMA: dst partition r, block q <- src row q*128+r
        src_v = xr.rearrange("(q p) c -> p q c", p=128)
        dst_v = xall[:].rearrange("p (q c) -> p q c", c=2 * L)
        nc.sync.dma_start(dst_v, src_v)
        xv = xall[:].rearrange("p (q l c) -> p q l c", q=NT, c=2)

        # dxy[:, c, q, t] = x[q, t+1, c] - x[q, t, c], both coords in one op
        dxy = fin.tile([128, 2 * NT * T], F32, tag="dxy")
        dxy4 = dxy[:].rearrange("p (c q t) -> p c q t", c=2, t=T)
        src_hi = bass.AP(xv.tensor, xv.offset + 2,
                         [xv.ap[0], [1, 2], [2 * L, NT], [2, T]])
        src_lo = bass.AP(xv.tensor, xv.offset,
                         [xv.ap[0], [1, 2], [2 * L, NT], [2, T]])
        nc.vector.tensor_tensor(dxy4, src_hi, src_lo, ALU.subtract)
        sq = fin.tile([128, 2 * NT * T], F32, tag="sq")
        nc.vector.tensor_tensor(sq[:], dxy[:], dxy[:], ALU.mult)
        ss = fin.tile([128, NT * T], F32, tag="ss")
        nc.vector.tensor_tensor(ss[:], sq[:, 0:NT * T], sq[:, NT * T:],
                                ALU.add)
        # v in bf16 (feeds the GRU and the angle ratio; ratio errors cancel)
        vbf = fkeep.tile([128, NT * T], BF16, tag="vbf")
        nc.scalar.activation(vbf[:], ss[:], AF.Sqrt)
        v3 = vbf[:].rearrange("p (q t) -> p q t", t=T)

        # veps = v + eps; den = pv_eps * v_eps via shifted views; s = pv + v
        veps = fkeep.tile([128, NT * T], BF16, tag="veps")
        nc.vector.tensor_scalar_add(veps[:], vbf[:], EPS)
        ve3 = veps[:].rearrange("p (q t) -> p q t", t=T)
        den = fkeep.tile([128, NT * T], BF16, tag="den")
        dn3 = den[:].rearrange("p (q t) -> p q t", t=T)
        nc.vector.tensor_tensor(dn3[:, :, 1:], ve3[:, :, 1:], ve3[:, :, :-1],
                                ALU.mult)
        nc.vector.tensor_tensor(dn3[:, :, 0:1], ve3[:, :, 0:1],
                                ve3[:, :, 0:1], ALU.mult)
        rden = fkeep.tile([128, NT * T], BF16, tag="rden")
        with nc.allow_low_precision("angle ratio; bf16 rel err ~0.4% on a "
                                    "~1e-3 rad feature is negligible"):
            nc.vector.reciprocal(rden[:], den[:])
        s = fkeep.tile([128, NT * T], BF16, tag="s")
        s3 = s[:].rearrange("p (q t) -> p q t", t=T)
        nc.vector.tensor_tensor(s3[:, :, 1:], v3[:, :, 1:], v3[:, :, :-1],
                                ALU.add)
        nc.vector.tensor_tensor(s3[:, :, 0:1], v3[:, :, 0:1], v3[:, :, 0:1],
                                ALU.add)
        # ang = sqrt(2*eps*(s+eps)*rden)
        nm = fkeep.tile([128, NT * T], BF16, tag="nm")
        nc.vector.scalar_tensor_tensor(nm[:], s[:], EPS, rden[:], ALU.add,
                                       ALU.mult)
        abf = fkeep.tile([128, NT * T], BF16, tag="abf")
        nc.scalar.activation(abf[:], nm[:], AF.Sqrt, scale=2.0 * EPS)

        onesb = fkeep.tile([128, R], BF16, tag="onesb")
        nc.vector.memset(onesb[:], 1.0)

        vt = fkeep.tile([T, R], BF16, tag="vt")
        at = fkeep.tile([T, R], BF16, tag="at")
        for p in range(NT):
            for src, dst in ((vbf, vt), (abf, at)):
                ps = fps.tile([T, 128], BF16, tag="tp")
                nc.tensor.transpose(ps[:], src[:, p * T:(p + 1) * T],
                                    ident[:])
                nc.vector.tensor_copy(dst[:, p * 128:(p + 1) * 128], ps[:])

        nc.sync.dma_start(f3[:, 0, :], onesb[0:T, :])
        nc.sync.dma_start(f3[:, 1, :], vt[:])
        nc.sync.dma_start(f3[:, 2, :], at[:])


def _build_gru_gat(nc, tc, f3, whhT_d, wih_d, bhhn_d, ident, ones, uwd_d,
                   wgT_d, gbias_d, out_d, t_steps=T, skip_gat=False):
    with (
        tc.tile_pool(name="wpool", bufs=1) as wpool,
        tc.tile_pool(name="hpool", bufs=2) as hpool,
    ):
        whhT = wpool.tile([HID, 3 * HID], BF16, tag="whhT")
        nc.sync.dma_start(whhT[:], whhT_d.ap())
        wih = wpool.tile([35, 3 * HID], BF16, tag="wih")
        nc.sync.dma_start(wih[:], wih_d.ap())
        bhhn = wpool.tile([33, HID], BF16, tag="bhhn")
        nc.sync.dma_start(bhhn[:], bhhn_d.ap())

        h_final = _gru(nc, tc, f3, whhT, wih, bhhn, ident, ones, hpool,
                       t_steps)
        if not skip_gat:
            _gat(nc, tc, h_final, uwd_d, wgT_d, gbias_d, ident, ones, out_d)
        else:
            osb = wpool.tile([BC, CD], F32, tag="osb_dbg")
            nc.vector.tensor_copy(osb[:], h_final[0][0:BC, 0:CD])
            nc.sync.dma_start(out_d.ap(), osb[:])


def _gru(nc, tc, f3, whhT, wih, bhhn, ident, ones, hpool, t_steps=T):
    """GRU steps over h [128 hid, 2048 rows] bf16, 4 row-streams."""
    with (
        tc.tile_pool(name="fpool", bufs=6) as fpool,
        tc.tile_pool(name="gru_sb", bufs=2 * NSTREAM) as gsb,
        tc.tile_pool(name="ps_rz", bufs=2, space="PSUM") as ps_rz,
        tc.tile_pool(name="ps_nh", bufs=2, space="PSUM") as ps_nh,
        tc.tile_pool(name="ps_gx", bufs=2, space="PSUM") as ps_gx,
    ):
        hs = []
        for s in range(NSTREAM):
            h0 = hpool.tile([HID, SC], BF16, tag=f"h{s}")
            nc.vector.memset(h0[:], 0.0)
            hs.append(h0)

        TB = 4  # timesteps per f-block DMA
        ftb = None
        for t in range(t_steps):
            # f rows (1, v_t, a_t) at partition bases 0 and 32 so stream
            # pairs can run K<=3 matmuls in distinct PE row groups.
            if t % TB == 0:
                nb = min(TB, t_steps - t)
                ftb = fpool.tile([35, TB * R], BF16, tag="ft")
                src = f3[t:t + nb].rearrange("t k r -> k t r")
                d0 = ftb[0:3, 0:nb * R].rearrange("k (t r) -> k t r", r=R)
                d1 = ftb[32:35, 0:nb * R].rearrange("k (t r) -> k t r", r=R)
                nc.sync.dma_start(d0, src)
                nc.sync.dma_start(d1, src)
            toff = (t % TB) * R
            ft = ftb[:, toff:toff + R]
            for pair in range(NSTREAM // 2):
                ss = (2 * pair, 2 * pair + 1)
                sls = [slice(s * SC, (s + 1) * SC) for s in ss]
                przs, pnhs, pgxs = [], [], []
                # packed ih matmuls first: only depend on ft
                for i, s in enumerate(ss):
                    bp = 32 * i if PACK_PAIRS else 0
                    prz = ps_rz.tile([128, 2 * SC], F32, tag="prz")
                    pnh = ps_nh.tile([128, SC], F32, tag="pnh")
                    pgx = ps_gx.tile([128, SC], F32, tag="pgx")
                    przs.append(prz); pnhs.append(pnh); pgxs.append(pgx)
                    nc.tensor.matmul(prz[:, 0:SC], wih[bp:bp + 3, 0:128],
                                     ft[bp:bp + 3, sls[i]],
                                     start=True, stop=False)
                    nc.tensor.matmul(prz[:, SC:], wih[bp:bp + 3, 128:256],
                                     ft[bp:bp + 3, sls[i]],
                                     start=True, stop=False)
                    nc.tensor.matmul(pgx[:], wih[bp:bp + 3, 256:384],
                                     ft[bp:bp + 3, sls[i]],
                                     start=True, stop=False)
                    nc.tensor.matmul(pnh[:], bhhn[bp:bp + 1, :],
                                     ft[bp:bp + 1, sls[i]],
                                     start=True, stop=False)
                for i, s in enumerate(ss):
                    prz, pnh, pgx = przs[i], pnhs[i], pgxs[i]
                    h_old = hs[s]
                    nc.tensor.matmul(prz[:, 0:SC], whhT[:, 0:128], h_old[:],
                                     start=False, stop=True)
                    nc.tensor.matmul(prz[:, SC:], whhT[:, 128:256], h_old[:],
                                     start=False, stop=True)
                    nc.tensor.matmul(pnh[:], whhT[:, 256:384], h_old[:],
                                     start=False, stop=True)
                    rz = gsb.tile([128, 2 * SC], BF16, tag="rz")
                    nc.scalar.activation(rz[:], prz[:], AF.Sigmoid)
                    t2 = gsb.tile([128, SC], BF16, tag="t2")
                    nc.vector.tensor_tensor(t2[:], rz[:, 0:SC], pnh[:],
                                            ALU.mult)
                    # accumulate r*gh_n onto the input part, tanh from PSUM
                    nc.tensor.matmul(pgx[:], ident[:], t2[:],
                                     start=False, stop=True)
                    nn = gsb.tile([128, SC], BF16, tag="nn")
                    nc.scalar.activation(nn[:], pgx[:], AF.Tanh)

                    d = gsb.tile([128, SC], BF16, tag="d")
                    nc.vector.tensor_tensor(d[:], h_old[:], nn[:],
                                            ALU.subtract)
                    nc.vector.tensor_tensor(d[:], rz[:, SC:], d[:], ALU.mult)
                    h_new = hpool.tile([HID, SC], BF16, tag=f"h{s}")
                    nc.vector.tensor_tensor(h_new[:], nn[:], d[:], ALU.add)
                    hs[s] = h_new
            h = hs
        return hs


def _gat(nc, tc, hs, uwd_d, wgT_d, gbias_d, ident, ones, out_d):
    """Attention from node 0 over all nodes, per batch of 128 rows.

    hs: list of NSTREAM tiles [HID, SC]; stream s holds rows [s*SC,(s+1)*SC),
    i.e. batches [4s, 4s+4).
    """
    with tc.tile_pool(name="gat_sb", bufs=1) as gsb:
        uwd = gsb.tile([HID, 2 * HEADS], BF16, tag="uwd")
        nc.sync.dma_start(uwd[:], uwd_d.ap())
        wgT = gsb.tile([HID, HEADS * CD], BF16, tag="wgT")
        nc.sync.dma_start(wgT[:], wgT_d.ap())
        gbias = gsb.tile([1, CD], BF16, tag="gbias")
        nc.sync.dma_start(gbias[:], gbias_d.ap())

        e = gsb.tile([HEADS, R], F32, tag="e")
        with tc.tile_pool(name="gat_ps", bufs=1, space="PSUM") as gps:
            # ssd[h, row] = <xh_row, u_h> ; dsd[h, row] = <xh_row, w_h>
            ssd = gps.tile([HEADS, R], F32, tag="ssd")
            dsd = gps.tile([HEADS, R], F32, tag="dsd")
            for c in range(R // SC):
                cs = slice(c * SC, (c + 1) * SC)
                nc.tensor.matmul(ssd[:, cs], uwd[:, 0:HEADS], hs[c][:],
                                 start=True, stop=True)
                nc.tensor.matmul(dsd[:, cs], uwd[:, HEADS:2 * HEADS],
                                 hs[c][:], start=True, stop=True)
            dsb = gsb.tile([HEADS, R], F32, tag="dsb")
            nc.vector.tensor_copy(dsb[:], dsd[:])

            # e[h, b*128+j] = s[h,b*128+j] + d[h, b*128] (attention logits)
            # d at node 0 per block, broadcast along j via a stride-0 AP.
            d0 = dsb[:].rearrange("h (b j) -> h b j", j=N)[:, :, 0:1]
            d0b = bass.AP(d0.tensor, d0.offset, list(d0.ap)[:-1] + [[0, N]])
            nc.vector.tensor_tensor(
                e[:].rearrange("h (b j) -> h b j", j=N),
                ssd[:].rearrange("h (b j) -> h b j", j=N), d0b, ALU.add)
        lr = gsb.tile([HEADS, R], F32, tag="lr")
        nc.scalar.activation(lr[:], e[:], AF.Lrelu, alpha=NEG_SLOPE)
        p = gsb.tile([HEADS, R], BF16, tag="p")
        nc.scalar.activation(p[:], lr[:], AF.Exp)

        # softmax denominators per (head, batch)
        ssum = gsb.tile([HEADS, BC], F32, tag="ssum")
        nc.vector.tensor_reduce(ssum[:], p[:].rearrange("h (b j) -> h b j",
                                                        j=N), AX.X, ALU.add)
        srec = gsb.tile([HEADS, BC], F32, tag="srec")
        nc.vector.reciprocal(srec[:], ssum[:])
        palpha = gsb.tile([HEADS, R], BF16, tag="palpha")
        s0 = srec[:]
        s0b = bass.AP(s0.tensor, s0.offset, list(s0.ap) + [[0, N]])
        nc.vector.tensor_tensor(
            palpha[:].rearrange("h (b j) -> h b j", j=N),
            p[:].rearrange("h (b j) -> h b j", j=N), s0b, ALU.mult)

        # transpose alpha and h per batch; ctx[f, (b h)] = sum_j hT[j,f]*aT[j,h]
        with tc.tile_pool(name="gat_ps2", bufs=2, space="PSUM") as gps2:
            pt = gsb.tile([128, HEADS * BC], BF16, tag="pt")
            ht = gsb.tile([128, R], BF16, tag="ht")
            ctx = gps2.tile([128, HEADS * BC], F32, tag="ctx")
            for b in range(BC):
                bs = slice(b * N, (b + 1) * N)
                lbs = slice((b % 4) * N, (b % 4 + 1) * N)
                pps = gps2.tile([128, HEADS], BF16, tag="pps")
                nc.tensor.transpose(pps[:], palpha[:, bs],
                                    ident[0:HEADS, 0:HEADS])
                nc.vector.tensor_copy(pt[:, b * HEADS:(b + 1) * HEADS],
                                      pps[:])
                nc.sync.dma_start_transpose(ht[:, bs], hs[b // 4][:, lbs])
            for b in range(BC):
                bs = slice(b * N, (b + 1) * N)
                nc.tensor.matmul(ctx[:, b * HEADS:(b + 1) * HEADS],
                                 ht[:, bs],
                                 pt[:, b * HEADS:(b + 1) * HEADS],
                                 start=True, stop=True)
            ctxs = gsb.tile([128, HEADS * BC], BF16, tag="ctxs")
            nc.vector.tensor_copy(ctxs[:], ctx[:])

            # out[b, c] = sum_h (W_h/4) ctx_bh + bias
            op = gps2.tile([BC, CD], F32, tag="op")
            ctx4 = ctxs[:].rearrange("f (b h) -> f h b", h=HEADS)
            for hh in range(HEADS):
                nc.tensor.matmul(op[:], ctx4[:, hh, :],
                                 wgT[:, hh * CD:(hh + 1) * CD],
                                 start=(hh == 0), stop=False)
            nc.tensor.matmul(op[:], ones[:, 0:BC], gbias[:], start=False,
                             stop=True)
            osb = gsb.tile([BC, CD], F32, tag="osb")
            nc.vector.tensor_copy(osb[:], op[:])
            nc.sync.dma_start(out_d.ap(), osb[:])


_NC_CACHE = None


def _get_program():
    global _NC_CACHE
    if _NC_CACHE is None:
        _NC_CACHE = _build_program()
    return _NC_CACHE


def prep_in_maps(x, gru_wih, gru_whh, gru_bih, gru_bhh, gat_w, gat_att_src,
                 gat_att_dst, gat_bias):
    x = np.asarray(x, np.float32)
    gru_wih = np.asarray(gru_wih, np.float32)
    gru_whh = np.asarray(gru_whh, np.float32)
    gru_bih = np.asarray(gru_bih, np.float32)
    gru_bhh = np.asarray(gru_bhh, np.float32)
    gat_w = np.asarray(gat_w, np.float32)
    gat_att_src = np.asarray(gat_att_src, np.float32)
    gat_att_dst = np.asarray(gat_att_dst, np.float32)
    gat_bias = np.asarray(gat_bias, np.float32)

    bf = ml_dtypes.bfloat16

    whhT = np.ascontiguousarray(gru_whh.T).astype(bf)  # [128, 384]
    # ih lhsT rows (bias, wv, wa) replicated at partition bases {0, 32};
    # bias = bih+bhh for r,z gates, bih only for n (bhh_n enters via r*gh_n).
    bias3 = gru_bih + gru_bhh
    bias3 = bias3.copy()
    bias3[2 * HID:] = gru_bih[2 * HID:]
    blk = np.stack([bias3, gru_wih[:, 0], gru_wih[:, 1]])  # [3, 384]
    wih_aug = np.zeros((35, 3 * HID), np.float32)
    wih_aug[0:3] = blk
    wih_aug[32:35] = blk
    wih_aug = wih_aug.astype(bf)
    bhh_n = np.zeros((33, HID), np.float32)
    bhh_n[0] = gru_bhh[2 * HID:]
    bhh_n[32] = gru_bhh[2 * HID:]
    bhh_n = bhh_n.astype(bf)
    ident = np.eye(128, dtype=np.float32).astype(bf)

    W = gat_w.reshape(HEADS, CD, CD)  # [h, c, f]
    u = np.einsum("hcf,hc->hf", W, gat_att_src)
    w = np.einsum("hcf,hc->hf", W, gat_att_dst)
    uwd = np.ascontiguousarray(np.concatenate([u, w], 0).T).astype(bf)
    # per-head lhsT [f, c] of W_h/HEADS, laid side by side -> [128, 512]
    wgT = np.ascontiguousarray(
        np.concatenate([(W[h] / HEADS).T for h in range(HEADS)], axis=1)
    ).astype(bf)
    gbias = gat_bias.reshape(1, CD).astype(bf)

    shared = dict(whhT=whhT, wih_aug=wih_aug, bhh_n=bhh_n, ident=ident,
                  uwd=uwd, wgT=wgT, gbias=gbias)
    in_maps = []
    for c in range(N_CORES):
        xc = x[c * BC:(c + 1) * BC].reshape(R, 2 * L)
        in_maps.append({"xr": np.ascontiguousarray(xc), **shared})
    return in_maps


def kernel(x, gru_wih, gru_whh, gru_bih, gru_bhh, gat_w, gat_att_src,
           gat_att_dst, gat_bias):
    in_maps = prep_in_maps(x, gru_wih, gru_whh, gru_bih, gru_bhh, gat_w,
                           gat_att_src, gat_att_dst, gat_bias)
    nc = _get_program()
    res = run_bass_kernel_spmd(nc, in_maps, list(range(N_CORES)))
    out = np.concatenate([res.results[c]["out"] for c in range(N_CORES)], 0)
    return out.astype(np.float32)

